# revision 2
# baseline (speedup 1.0000x reference)
"""Fused linear + cross-entropy loss on 8 Trainium2 NeuronCores.

Vocab-parallel fp8 DoubleRow kernel, restructured from the baseline around
the LDWEIGHTS bottleneck: the baseline emitted one LDWEIGHTS per matmul
(3328 each; LDW stream ~709us vs matmul stream ~771us on the PE's two SBUF
read ports — zero slack for the load-ahead pipeline). v2 shares one
LDWEIGHTS across all matmuls with the same stationary operand via a
post-Tile dedup pass (drop waitless InstLdweights whose weights AP matches
the previously loaded one), cutting LDW count 3.25x to 1024 so weight
loads fully hide under matmul streaming.

Structure per core (V_LOC = 6283 vocab cols = ceil(50257/8), 4096 tokens,
d = 2048; padding trimmed from the baseline's 6400 for 1.8% less work):
  - x^T resident in SBUF as 8 k-pair tiles [128, 2, 4096] fp8
  - W^T streamed per vocab chunk (vc outer, t inner); chunks
    [2048, 2048, 1536, 651], each chunk's W split in 2 k-half tiles;
    every chunk keeps >=2 matmuls (>213ns streaming) per LDWEIGHTS so the
    weight-load pipeline always hides
  - loop: for vc: for t(32): for j(8): 2-4 matmuls of <=512 cols,
    stationary x(t,j), into a 4-bank PSUM tile (2 rotating tiles)
  - ScalarE drains each (vc,t) tile: exp + row-sum via accum_out into
    sumexp[128, vc*32+t]; the exp'd tile goes to a rotating SBUF scratch,
    never read back
  - picked logits are NOT computed on device: the host gathers
    picked[i] = x[i] . weight[label_i] in exact fp32 (8 MFLOP)

Host merge: lse = log(sum_cores sumexp - n_pad), loss = mean(lse - picked)
+ 1e-4 * mean(lse^2). No collectives.
"""

import numpy as np
import ml_dtypes

import concourse.bass as bass
import concourse.mybir as mybir
import concourse.tile as tile
from concourse import bacc
from concourse.bass_utils import run_bass_kernel_spmd

# ---- problem constants (hardcoded per contract) ----
D = 2048            # in_features (contraction)
V_TOTAL = 50257     # vocab
N_CORES = 8
V_LOC = 6283        # padded per-core vocab shard (8*6283 = 50264)
N_PAD = N_CORES * V_LOC - V_TOTAL  # 7 zero rows, all on core 7
N_TOK = 4096        # 2*2048 tokens
KT = D // 128       # 16 contraction chunks (8 DoubleRow k-pairs)
# PSUM-tile sized vocab chunks; every chunk keeps >=2 matmuls (>213ns of
# streaming) per LDWEIGHTS so the weight-load pipeline always hides
V_CHUNKS = [2048, 2048, 1536, 651]
IGNORE_INDEX = -100
Z_REG = 1e-4
W_SCALE = 64.0      # fp8: W pre-scale (power of 2: exact to undo)
F32 = mybir.dt.float32
BF16 = mybir.dt.bfloat16
FP8 = mybir.dt.float8e4
NP_FP8 = mybir.dt.np(FP8)          # ml_dtypes.float8_e4m3
DR = mybir.MatmulPerfMode.DoubleRow


def dedup_ldweights(nc):
    """Drop InstLdweights that reload the PE array with the weights it
    already holds. Tile emits one LDW per matmul; consecutive matmuls with
    the same stationary operand only need the first. Only waitless,
    update-free LDWs are dropped (sync-carrying ones keep their role)."""
    removed = 0
    for blk in nc.main_func.blocks:
        cur = None
        keep = []
        dropped = False
        for inst in blk.instructions:
            if isinstance(inst, mybir.InstLdweights):
                sig = (str(inst.ins[0]), str(inst.perf_mode),
                       str(inst.is_transpose), str(inst.tile_position))
                si = inst.sync_info
                has_sync = si is not None and (
                    len(si.on_wait) > 0 or len(si.on_update) > 0
                )
                if cur == sig and not has_sync:
                    removed += 1
                    dropped = True
                    continue
                cur = sig
            keep.append(inst)
        if dropped:
            blk.instructions[:] = keep
    return removed


def build_nc(n_tok=N_TOK, v_chunks=None, repeats=1, dedup=True):
    """Build the per-core Bass program (same program on all 8 cores)."""
    if v_chunks is None:
        v_chunks = V_CHUNKS
    v_loc = sum(v_chunks)
    t_tiles = n_tok // 128
    nvc = len(v_chunks)
    ncols = nvc * t_tiles
    gmax = max(v_chunks)

    nc = bacc.Bacc(None, target_bir_lowering=False, debug=False)

    xT = nc.dram_tensor("xT", [D, n_tok], FP8, kind="ExternalInput")
    wT = nc.dram_tensor("wT", [D, v_loc], FP8, kind="ExternalInput")
    sumexp = nc.dram_tensor("sumexp", [128, ncols], F32, kind="ExternalOutput")

    xT_r = xT.rearrange("(k p) n -> p k n", p=128)   # [128, KT, n_tok]
    wT_r = wT.rearrange("(k p) v -> p k v", p=128)   # [128, KT, v_loc]

    with tile.TileContext(nc) as tc:
        with (
            tc.tile_pool(name="xpool", bufs=1) as xpool,
            tc.tile_pool(name="wpool", bufs=4) as wpool,
            tc.tile_pool(name="cpool", bufs=1) as cpool,
            tc.tile_pool(name="spool", bufs=2) as spool,
            tc.tile_pool(name="ppool", bufs=2, space=bass.MemorySpace.PSUM) as ppool,
        ):
            # W chunk 0 first in the DMA queue (gates the first matmuls);
            # each chunk is 2 tiles of 4 k-pairs so the j-loop can start
            # once the first half lands.
            def dma_w(voff, vcols):
                tiles = []
                for h in range(2):
                    wt = wpool.tile([128, 8, gmax], FP8, tag=f"w{h}")
                    nc.sync.dma_start(
                        out=wt[:, :, :vcols],
                        in_=wT_r[:, 8 * h : 8 * h + 8, voff : voff + vcols],
                    )
                    tiles.append(wt)
                return tiles

            w_tiles = dma_w(0, v_chunks[0])

            # x: 8 independent k-pair tiles so compute starts as they land
            x_pairs = []
            for j in range(KT // 2):
                xp = xpool.tile([128, 2, n_tok], FP8, tag=f"x{j}")
                nc.sync.dma_start(out=xp[:], in_=xT_r[:, 2 * j : 2 * j + 2, :])
                x_pairs.append(xp)

            se_acc = cpool.tile([128, ncols], F32)

            for _rep in range(repeats):
                voff = 0
                for vci, vcols in enumerate(v_chunks):
                    if not (vci == 0 and _rep == 0):
                        w_tiles = dma_w(voff, vcols)
                    nbank = (vcols + 511) // 512
                    for t in range(t_tiles):
                        ps = ppool.tile([128, gmax], F32, tag="ps")
                        tok = slice(t * 128, (t + 1) * 128)
                        for j in range(KT // 2):
                            wt = w_tiles[j // 4]
                            kk = slice(2 * (j % 4), 2 * (j % 4) + 2)
                            for b in range(nbank):
                                vs = min(512, vcols - b * 512)
                                bank = slice(b * 512, b * 512 + vs)
                                nc.tensor.matmul(
                                    ps[:, bank],
                                    x_pairs[j][:, :, tok],
                                    wt[:, kk, bank],
                                    start=(j == 0),
                                    stop=(j == KT // 2 - 1),
                                    perf_mode=DR,
                                    skip_group_check=True,
                                )
                        col = vci * t_tiles + t
                        ex = spool.tile([128, gmax], BF16, tag="ex")
                        nc.scalar.activation(
                            ex[:, :vcols],
                            ps[:, :vcols],
                            mybir.ActivationFunctionType.Exp,
                            scale=1.0 / W_SCALE,
                            accum_out=se_acc[:, col : col + 1],
                        )
                    voff += vcols

            nc.sync.dma_start(out=sumexp[:], in_=se_acc[:])

    if dedup:
        n = dedup_ldweights(nc)
        assert n > 0
    nc.compile()
    return nc


def make_in_maps(x, labels, weight, n_tok=N_TOK, v_chunks=None, n_cores=N_CORES):
    """Host-side prep: transpose/cast inputs, build per-core input maps."""
    if v_chunks is None:
        v_chunks = V_CHUNKS
    v_loc = sum(v_chunks)

    xf = np.ascontiguousarray(x.reshape(n_tok, D).T).astype(NP_FP8)
    wb = (weight * np.float32(W_SCALE)).astype(NP_FP8)
    v_total = weight.shape[0]
    wpad = np.zeros((n_cores * v_loc, D), NP_FP8)
    wpad[:v_total] = wb

    in_maps = []
    for m in range(n_cores):
        wT_m = np.ascontiguousarray(wpad[m * v_loc : (m + 1) * v_loc].T)
        in_maps.append({"xT": xf, "wT": wT_m})
    return in_maps


def merge_results(results, x, labels, weight, n_tok=N_TOK, v_chunks=None,
                  n_pad=N_PAD):
    """Host-side merge: lse from device sumexp; picked gathered in fp32."""
    if v_chunks is None:
        v_chunks = V_CHUNKS
    t_tiles = n_tok // 128
    nvc = len(v_chunks)
    se = np.stack([np.asarray(r["sumexp"], np.float64) for r in results])
    # [cores, 128, nvc, T] -> per-token [cores, n_tok] (token = t*128 + p)
    se_tok = (
        se.reshape(-1, 128, nvc, t_tiles).sum(2).transpose(0, 2, 1).reshape(-1, n_tok)
    )
    sumexp_tok = se_tok.sum(0) - float(n_pad)  # padding rows give exp(0)=1 each

    lab_flat = labels.reshape(-1).astype(np.int64)
    valid = lab_flat != IGNORE_INDEX
    n_valid = float(valid.sum())
    denom = max(n_valid, 1.0)
    safe_labels = np.where(valid, lab_flat, 0)
    x_flat = x.reshape(-1, D).astype(np.float32)
    picked = np.einsum("nd,nd->n", x_flat, weight[safe_labels].astype(np.float32))
    lse = np.log(sumexp_tok)
    nll = lse - picked
    loss = np.where(valid, nll, 0.0).sum() / denom
    if Z_REG > 0.0 and n_valid > 0:
        loss = loss + Z_REG * np.where(valid, lse * lse, 0.0).sum() / denom
    return np.float32(loss)


_CACHE = {}


def kernel(x, labels, weight):
    x = np.asarray(x, dtype=np.float32)
    labels_np = np.asarray(labels)
    weight = np.asarray(weight, dtype=np.float32)

    if "nc" not in _CACHE:
        _CACHE["nc"] = build_nc()
    nc = _CACHE["nc"]

    in_maps = make_in_maps(x, labels_np, weight)
    res = run_bass_kernel_spmd(nc, in_maps, core_ids=list(range(N_CORES)))
    return merge_results(res.results, x, labels_np, weight)


# revision 3
# speedup vs baseline: 1.5775x; 1.5775x over previous
"""Fused linear + cross-entropy loss on 8 Trainium2 NeuronCores.

Vocab-parallel fp8 DoubleRow kernel, restructured from the baseline around
the LDWEIGHTS bottleneck: the baseline emitted one LDWEIGHTS per matmul
(3328 each; LDW stream ~709us vs matmul stream ~771us on the PE's two SBUF
read ports — zero slack for the load-ahead pipeline). v2 shares one
LDWEIGHTS across all matmuls with the same stationary operand via a
post-Tile dedup pass (drop waitless InstLdweights whose weights AP matches
the previously loaded one), cutting LDW count 3.25x to 1024 so weight
loads fully hide under matmul streaming.

Structure per core (V_LOC = 6283 vocab cols = ceil(50257/8), 4096 tokens,
d = 2048; padding trimmed from the baseline's 6400 for 1.8% less work):
  - x^T resident in SBUF as 8 k-pair tiles [128, 2, 4096] fp8
  - W^T streamed per vocab chunk (vc outer, t inner); chunks
    [2048, 2048, 1536, 651], each chunk's W split in 2 k-half tiles;
    every chunk keeps >=2 matmuls (>213ns streaming) per LDWEIGHTS so the
    weight-load pipeline always hides
  - loop: for vc: for t(32): for j(8): 2-4 matmuls of <=512 cols,
    stationary x(t,j), into a 4-bank PSUM tile (2 rotating tiles)
  - ScalarE drains each (vc,t) tile: exp + row-sum via accum_out into
    sumexp[128, vc*32+t]; the exp'd tile goes to a rotating SBUF scratch,
    never read back
  - picked logits are NOT computed on device: the host gathers
    picked[i] = x[i] . weight[label_i] in exact fp32 (8 MFLOP)

Host merge: lse = log(sum_cores sumexp - n_pad), loss = mean(lse - picked)
+ 1e-4 * mean(lse^2). No collectives.
"""

import numpy as np
import ml_dtypes

import concourse.bass as bass
import concourse.mybir as mybir
import concourse.tile as tile
from concourse import bacc
from concourse.bass_utils import run_bass_kernel_spmd

# ---- problem constants (hardcoded per contract) ----
D = 2048            # in_features (contraction)
V_TOTAL = 50257     # vocab
N_CORES = 8
V_LOC = 6283        # padded per-core vocab shard (8*6283 = 50264)
N_PAD = N_CORES * V_LOC - V_TOTAL  # 7 zero rows, all on core 7
N_TOK = 4096        # 2*2048 tokens
KT = D // 128       # 16 contraction chunks (8 DoubleRow k-pairs)
# PSUM-tile sized vocab chunks; every chunk keeps >=2 matmuls (>213ns of
# streaming) per LDWEIGHTS so the weight-load pipeline always hides
V_CHUNKS = [2048, 2048, 1536, 651]
IGNORE_INDEX = -100
Z_REG = 1e-4
W_SCALE = 64.0      # fp8: W pre-scale (power of 2: exact to undo)
F32 = mybir.dt.float32
BF16 = mybir.dt.bfloat16
FP8 = mybir.dt.float8e4
NP_FP8 = mybir.dt.np(FP8)          # ml_dtypes.float8_e4m3
DR = mybir.MatmulPerfMode.DoubleRow


def dedup_ldweights(nc):
    """Drop InstLdweights that reload the PE array with the weights it
    already holds. Tile emits one LDW per matmul; consecutive matmuls with
    the same stationary operand only need the first. Only waitless,
    update-free LDWs are dropped (sync-carrying ones keep their role)."""
    removed = 0
    for blk in nc.main_func.blocks:
        cur = None
        keep = []
        dropped = False
        for inst in blk.instructions:
            if isinstance(inst, mybir.InstLdweights):
                sig = (str(inst.ins[0]), str(inst.perf_mode),
                       str(inst.is_transpose), str(inst.tile_position))
                si = inst.sync_info
                has_sync = si is not None and (
                    len(si.on_wait) > 0 or len(si.on_update) > 0
                )
                if cur == sig and not has_sync:
                    removed += 1
                    dropped = True
                    continue
                cur = sig
            keep.append(inst)
        if dropped:
            blk.instructions[:] = keep
    return removed


def build_nc(n_tok=N_TOK, v_chunks=None, repeats=1, dedup=True):
    """Build the per-core Bass program (same program on all 8 cores)."""
    if v_chunks is None:
        v_chunks = V_CHUNKS
    v_loc = sum(v_chunks)
    t_tiles = n_tok // 128
    nvc = len(v_chunks)
    ncols = nvc * t_tiles
    gmax = max(v_chunks)

    nc = bacc.Bacc(None, target_bir_lowering=False, debug=False)

    xT = nc.dram_tensor("xT", [D, n_tok], FP8, kind="ExternalInput")
    wT = nc.dram_tensor("wT", [D, v_loc], FP8, kind="ExternalInput")
    sumexp = nc.dram_tensor("sumexp", [128, ncols], F32, kind="ExternalOutput")

    xT_r = xT.rearrange("(k p) n -> p k n", p=128)   # [128, KT, n_tok]
    wT_r = wT.rearrange("(k p) v -> p k v", p=128)   # [128, KT, v_loc]

    with tile.TileContext(nc) as tc:
        with (
            tc.tile_pool(name="xpool", bufs=1) as xpool,
            tc.tile_pool(name="wpool", bufs=4) as wpool,
            tc.tile_pool(name="cpool", bufs=1) as cpool,
            tc.tile_pool(name="spool", bufs=2) as spool,
            tc.tile_pool(name="ppool", bufs=2, space=bass.MemorySpace.PSUM) as ppool,
        ):
            # W chunk 0 first in the DMA queue (gates the first matmuls);
            # each chunk is 2 tiles of 4 k-pairs so the j-loop can start
            # once the first half lands.
            def dma_w(voff, vcols):
                tiles = []
                for h in range(2):
                    wt = wpool.tile([128, 8, gmax], FP8, tag=f"w{h}")
                    nc.sync.dma_start(
                        out=wt[:, :, :vcols],
                        in_=wT_r[:, 8 * h : 8 * h + 8, voff : voff + vcols],
                    )
                    tiles.append(wt)
                return tiles

            w_tiles = dma_w(0, v_chunks[0])

            # x: 8 independent k-pair tiles so compute starts as they land
            x_pairs = []
            for j in range(KT // 2):
                xp = xpool.tile([128, 2, n_tok], FP8, tag=f"x{j}")
                nc.sync.dma_start(out=xp[:], in_=xT_r[:, 2 * j : 2 * j + 2, :])
                x_pairs.append(xp)

            se_acc = cpool.tile([128, ncols], F32)

            for _rep in range(repeats):
                voff = 0
                for vci, vcols in enumerate(v_chunks):
                    if not (vci == 0 and _rep == 0):
                        w_tiles = dma_w(voff, vcols)
                    nbank = (vcols + 511) // 512
                    for t in range(t_tiles):
                        ps = ppool.tile([128, gmax], F32, tag="ps")
                        tok = slice(t * 128, (t + 1) * 128)
                        for j in range(KT // 2):
                            wt = w_tiles[j // 4]
                            kk = slice(2 * (j % 4), 2 * (j % 4) + 2)
                            for b in range(nbank):
                                vs = min(512, vcols - b * 512)
                                bank = slice(b * 512, b * 512 + vs)
                                nc.tensor.matmul(
                                    ps[:, bank],
                                    x_pairs[j][:, :, tok],
                                    wt[:, kk, bank],
                                    start=(j == 0),
                                    stop=(j == KT // 2 - 1),
                                    perf_mode=DR,
                                    skip_group_check=True,
                                )
                        col = vci * t_tiles + t
                        ex = spool.tile([128, gmax], BF16, tag="ex")
                        nc.scalar.activation(
                            ex[:, :vcols],
                            ps[:, :vcols],
                            mybir.ActivationFunctionType.Exp,
                            scale=1.0 / W_SCALE,
                            accum_out=se_acc[:, col : col + 1],
                        )
                    voff += vcols

            nc.sync.dma_start(out=sumexp[:], in_=se_acc[:])

    if dedup:
        n = dedup_ldweights(nc)
        assert n > 0
    nc.compile()
    return nc


def make_in_maps(x, labels, weight, n_tok=N_TOK, v_chunks=None, n_cores=N_CORES):
    """Host-side prep: transpose/cast inputs, build per-core input maps."""
    if v_chunks is None:
        v_chunks = V_CHUNKS
    v_loc = sum(v_chunks)

    xf = np.ascontiguousarray(x.reshape(n_tok, D).T).astype(NP_FP8)
    wb = (weight * np.float32(W_SCALE)).astype(NP_FP8)
    v_total = weight.shape[0]
    wpad = np.zeros((n_cores * v_loc, D), NP_FP8)
    wpad[:v_total] = wb

    in_maps = []
    for m in range(n_cores):
        wT_m = np.ascontiguousarray(wpad[m * v_loc : (m + 1) * v_loc].T)
        in_maps.append({"xT": xf, "wT": wT_m})
    return in_maps


def merge_results(results, x, labels, weight, n_tok=N_TOK, v_chunks=None,
                  n_pad=N_PAD):
    """Host-side merge: lse from device sumexp; picked gathered in fp32."""
    if v_chunks is None:
        v_chunks = V_CHUNKS
    t_tiles = n_tok // 128
    nvc = len(v_chunks)
    se = np.stack([np.asarray(r["sumexp"], np.float64) for r in results])
    # [cores, 128, nvc, T] -> per-token [cores, n_tok] (token = t*128 + p)
    se_tok = (
        se.reshape(-1, 128, nvc, t_tiles).sum(2).transpose(0, 2, 1).reshape(-1, n_tok)
    )
    sumexp_tok = se_tok.sum(0) - float(n_pad)  # padding rows give exp(0)=1 each

    lab_flat = labels.reshape(-1).astype(np.int64)
    valid = lab_flat != IGNORE_INDEX
    n_valid = float(valid.sum())
    denom = max(n_valid, 1.0)
    safe_labels = np.where(valid, lab_flat, 0)
    x_flat = x.reshape(-1, D).astype(np.float32)
    picked = np.einsum("nd,nd->n", x_flat, weight[safe_labels].astype(np.float32))
    lse = np.log(sumexp_tok)
    nll = lse - picked
    loss = np.where(valid, nll, 0.0).sum() / denom
    if Z_REG > 0.0 and n_valid > 0:
        loss = loss + Z_REG * np.where(valid, lse * lse, 0.0).sum() / denom
    return np.float32(loss)


_CACHE = {}


def kernel(x, labels, weight):
    x = np.asarray(x, dtype=np.float32)
    labels_np = np.asarray(labels)
    weight = np.asarray(weight, dtype=np.float32)

    if "nc" not in _CACHE:
        _CACHE["nc"] = build_nc()
    nc = _CACHE["nc"]

    in_maps = make_in_maps(x, labels_np, weight)
    try:
        res = run_bass_kernel_spmd(nc, in_maps, core_ids=list(range(N_CORES)))
    except Exception:
        # transient NRT_EXEC_UNIT_UNRECOVERABLE device faults: retry once
        import time
        time.sleep(2.0)
        res = run_bass_kernel_spmd(nc, in_maps, core_ids=list(range(N_CORES)))
    return merge_results(res.results, x, labels_np, weight)
